# revision 18
# baseline (speedup 1.0000x reference)
"""Trainium2 Bass kernel for nn_MessagePassing (ring GNN, 5 nodes, 18 hid).

Math (per batch element b, node n, ring of 5):
  h_n = tanh(x_n @ Wf + bf)                       x_n in R^6, h_n in R^18
  M_n = tanh(h_n @ Wm[:18] + h_{n+1} @ Wm[18:] + bm)
  U_n = tanh(M_{n-1} @ Wu[:18] + h_n @ (Wu[18:36]+Wu[36:54]) + bu)
  out = concat(U_0..U_4) @ Wr + br

Layout: feature-major — each batch element is one 90-row column
(5 nodes x 18 hid); batch runs along the free axis, 1024 columns per
superchunk (SC).  All weights are fp16; activations are fp16 in SBUF.

v1 change vs baseline: the readout matmul is FUSED into the f matmul.
The stationary weight block for the f stage carries one extra output
column (91) holding Wr over contraction rows 0-90 (the u values of SC
k-4, which live in rows 0-90 of the same rhs tile that carries x for
SC k in rows 96-126).  This removes the separate readout stream: PE
work drops from 5 to 4 1024-column streams per SC.  psum_h row 96 then
holds r[k-4] and is drained by a DVE row-copy into the store tile.

Engine plan per SC:
  PE    : f+r (rows 0-126 -> out 0-91), m (0-90 -> 0-89),
          uM+uh (accumulating pair -> out 0-95)
  ACT   : exact-table tanh of psum columns [0:ACT_COLS] for h/m/u
  DVE   : TANH7_ANT poly drain of columns [ACT_COLS:1024] + the
          [1,1024] r-row copy psum_h[91] -> store tile
  SP DMA: x in (1 SC per load, into rows 96-127 of the u tile of SC
          k-4), out (1 store per 8 SCs)

The loop runs N_SC+4 iterations: iterations 0-3 are f-only (no u yet;
separate head x tiles), iterations N_SC..N_SC+3 are r-only tail
matmuls (lhsT rows 0-90, so the x rows are never read).  The u drain
writes zeros into rows 91-95 (the wuh/wum blocks carry 5 zero output
columns) so the fused matmul never reads uninitialized partitions.

Biases ride the matmuls exactly as in the baseline: x carries a
ones-row (tile row 126) whose weight row holds bf; h and u carry a
carrier row (row 90, value tanh(CARRIER)) whose weight entries are
biases scaled by 1/tanh(CARRIER); br rides the fused readout column at
the u-carrier row.
"""

import os
import sys

import numpy as np

if not any(os.path.isdir(os.path.join(p, "concourse")) for p in sys.path if p):
    sys.path.insert(0, "/opt/trn_rl_repo")

N_HID = 18
NODES = 5
F_IN = 6
B = 262144
N_CORES = 8
B_CORE = B // N_CORES   # 32768
SC = 1024               # batch columns per superchunk
N_SC = B_CORE // SC     # 32
D90 = NODES * N_HID     # 90
D91 = D90 + 1

ACT_COLS = 301          # ACT's tanh share per layer; DVE takes the rest
# (ACT also owns the full second m k-tile: the DVE custom op cannot
# partition-rebase psum rows 64:128 onto sbuf rows 0:64, ACT can)
CARRIER = 1.2           # bias carrier pre-activation on h/u row 90

# degree-7 odd minimax tanh coefficients: x*(c0 + c1 s + c2 s^2 + c3 s^3)
CH = (0.99191345085891702, -0.29147337765012465,
      0.069278752318150422, -0.0073386055200310675)   # fit on [0, 1.8]
CM = (0.99880200804513142, -0.32174878994875622,
      0.10166490186270968, -0.017242076990493044)     # fit on [0, 1.25]

_TANH7 = None


def _poly_tanh_np(v, c):
    v = np.asarray(v, np.float32)
    s = v * v
    return v * (((np.float32(c[3]) * s + np.float32(c[2])) * s
                 + np.float32(c[1])) * s + np.float32(c[0]))


def _register_tanh7():
    """Register the TANH7_ANT custom DVE op (idempotent)."""
    global _TANH7
    if _TANH7 is not None:
        return _TANH7
    import concourse.dve_ops as dve_ops
    from concourse.dve_ops import DveOp, OPS, CUSTOM_DVE_SPECS, _SUB_OPCODE_FOR_NAME
    from concourse.dve_spec import Spec, Src0, C0, C1, C2, C3, lower, _spill_c3_to_src1
    from concourse.dve_uop import DveOpSpec

    name = "TANH7_ANT"
    for op in OPS:
        if op.name == name:
            _TANH7 = op
            return op

    # y = x * (((C2*s + C1)*s + C0)*s + C3),  s = x^2   [8 ALU ops exactly]
    # C0=c1 (s0), C1=c2 (s1), C2=c3 (imm2), C3=c0 (spilled via in1)
    s = Src0 * Src0
    body = Src0 * (((C2 * s + C1) * s + C0) * s + C3)
    body = _spill_c3_to_src1(body)

    def _ref(in0, in1, c0, c1, c2):
        in0 = np.asarray(in0, np.float32)
        s = in0 * in0
        c3v = np.asarray(in1, np.float32) if in1 is not None else np.float32(0)
        return in0 * (((np.float32(c2) * s + np.float32(c1)) * s
                       + np.float32(c0)) * s + c3v)

    spec = Spec(body=body, reference=_ref)

    from concourse.dve_ops import get_dve_sub_opcode  # noqa: F401
    row = max(_SUB_OPCODE_FOR_NAME.values()) + 1
    shas = {}
    for ver in ("v3", "v4"):
        tmp = DveOpSpec(name=name, opcode=row, uops=lower(spec, ver=ver),
                       rd1_en=True)
        shas[ver] = tmp.sha(ver)

    op = DveOp(name, spec, subdim=False, uops_sha=shas)
    OPS.append(op)
    CUSTOM_DVE_SPECS[name] = spec
    _SUB_OPCODE_FOR_NAME[name] = row
    _TANH7 = op
    return op


# c16 column map: 0:97 wfr | 97:225 wm (128 wide: 90 real + 38 zero, so
# psum_m rows 90-127 are computed zeros that pad the second fp8 k-tile) |
# 225:321 wuh.  wum lives in the separate fp8 tensor consts8 [64, 2, 96]
# (k-tile 0 = contraction rows 0-63, k-tile 1 = rows 64-89 + zero pad) for
# the DoubleRow uM matmul.
O_WM = 97
O_WUH = 225
N_CONST16 = 321


def _build_weight_blocks(Wf, bf, Wm, bm, Wu, bu, Wr, br):
    f32 = np.float32
    Wf = np.asarray(Wf, f32); bf = np.asarray(bf, f32)
    Wm = np.asarray(Wm, f32); bm = np.asarray(bm, f32)
    Wu = np.asarray(Wu, f32); bu = np.asarray(bu, f32)
    Wr = np.asarray(Wr, f32); br = np.asarray(br, f32)
    tc15 = float(np.tanh(CARRIER))

    c16 = np.zeros((128, N_CONST16), f32)
    # --- wfr: f rows 96..126 (x features + ones-row 126) -> cols 0..90,
    #     readout rows 0..90 (u of SC k-4) -> col 91 ---
    for n in range(NODES):
        for f in range(F_IN):
            c16[96 + F_IN * n + f, N_HID * n:N_HID * n + N_HID] = Wf[f]
        c16[126, N_HID * n:N_HID * n + N_HID] = bf
    c16[126, 90] = CARRIER  # carrier column -> psum_h row 90 = CARRIER
    c16[0:D90, 96] = Wr.reshape(D90)
    c16[90, 96] = br[0] / tc15  # br rides the u carrier row

    # --- wm: rows 0..90, 90 out cols; output col block n holds M_{n-1} ---
    o = O_WM
    for n in range(NODES):
        e = (n - 1) % NODES     # edge index: M_e = tanh(h_e Wm1 + h_{e+1} Wm2)
        c16[N_HID * e:N_HID * e + N_HID, o + N_HID * n:o + N_HID * n + N_HID] += Wm[:N_HID]
        c16[N_HID * ((e + 1) % NODES):N_HID * ((e + 1) % NODES) + N_HID,
            o + N_HID * n:o + N_HID * n + N_HID] += Wm[N_HID:]
        c16[90, o + N_HID * n:o + N_HID * n + N_HID] = bm / tc15

    # --- wuh: rows 0..90, 96 out cols (91..95 zero so psum_u rows 91..95
    #     are initialized; col 90 = carrier for br) ---
    o = O_WUH
    wuh = Wu[N_HID:2 * N_HID] + Wu[2 * N_HID:3 * N_HID]
    for n in range(NODES):
        c16[N_HID * n:N_HID * n + N_HID, o + N_HID * n:o + N_HID * n + N_HID] = wuh
        c16[90, o + N_HID * n:o + N_HID * n + N_HID] = bu / tc15
    c16[90, o + 90] = CARRIER / tc15  # psum_u row 90 = CARRIER

    # --- wum8: rows 0..89 (m_t is pre-rolled), 96 out cols, split into two
    #     fp8 k-tiles of 64 contraction rows for the DoubleRow uM matmul ---
    wum_full = np.zeros((128, 96), f32)
    for n in range(NODES):
        wum_full[N_HID * n:N_HID * n + N_HID,
                 N_HID * n:N_HID * n + N_HID] = Wu[:N_HID]
    wum8 = np.zeros((64, 2, 96), f32)
    wum8[:, 0, :] = wum_full[0:64]
    wum8[:, 1, :] = wum_full[64:128]

    # --- f32 consts: poly c0 per layer + br ---
    c32 = np.zeros((128, 3), f32)
    c32[:, 0] = CH[0]
    c32[:, 1] = CM[0]
    c32[:, 2] = br[0]
    import ml_dtypes
    return c16.astype(np.float16), c32, wum8.astype(ml_dtypes.float8_e4m3)


def _prep_core_x(xc):
    """[B_CORE, 30] -> [N_SC, 32, SC] fp16; row 30 = 1.0 (bias row)."""
    arr = np.zeros((N_SC, 32, SC), np.float32)
    arr[:, :30] = xc.reshape(N_SC, SC, 30).transpose(0, 2, 1)
    arr[:, 30] = 1.0
    return np.ascontiguousarray(arr).astype(np.float16)


def _split_multi_waits(nc, mybir):
    """walrus's per-instruction sync-wait encoding holds only one wait per
    opcode struct; hoist extra waits onto same-engine NoOps placed before
    the instruction."""
    n = 0
    for fn in nc.m.functions:
        for bb in fn.blocks:
            new_insts = []
            for inst in bb.instructions:
                si = inst.sync_info
                if si is not None and si.on_wait and len(si.on_wait) > 1:
                    waits = list(si.on_wait)
                    for w in waits[:-1]:
                        n += 1
                        nop = mybir.InstNoOp(name=f"I-waitnop-{n}", ins=[], outs=[])
                        nop.engine = inst.engine
                        nop.sync_info = mybir.SyncInfo(on_wait=[w], on_update=[])
                        nc.register_instruction(nop)
                        new_insts.append(nop)
                    inst.sync_info = mybir.SyncInfo(
                        on_wait=[waits[-1]], on_update=list(si.on_update or [])
                    )
                new_insts.append(inst)
            if n:
                bb.instructions = new_insts
    return n


def _lower_custom_dve(nc, mybir):
    """Pack InstCustomDveAnt into ISA bytes (the walrus flow expects the
    bytes pre-packed; only the Bacc flow runs codegen_inst_isa_subclasses)."""
    for fn in nc.m.functions:
        for bb in fn.blocks:
            new = []
            for inst in bb.instructions:
                if isinstance(inst, mybir.InstCustomDveAnt):
                    new.extend(mybir.codegen_inst_isa_one(inst, nc._state, nc.isa))
                else:
                    new.append(inst)
            bb.instructions = new


def _build_program(reps=1, act_cols=None, r_lag=4):
    import concourse.bass as bass
    import concourse.mybir as mybir
    from concourse.tile import TileContext

    tanh7 = _register_tanh7()

    f32 = mybir.dt.float32
    f16 = mybir.dt.float16
    Tanh = mybir.ActivationFunctionType.Tanh
    A = ACT_COLS if act_cols is None else act_cols

    f8 = mybir.dt.float8e4
    DR = mybir.MatmulPerfMode.DoubleRow
    nc = bass.Bass("TRN2")
    x_d = nc.dram_tensor("x_prep", [N_SC, 32, SC], f16, kind="ExternalInput")
    c16_d = nc.dram_tensor("consts16", [128, N_CONST16], f16, kind="ExternalInput")
    c8_d = nc.dram_tensor("consts8", [64, 2, 96], f8, kind="ExternalInput")
    c32_d = nc.dram_tensor("consts32", [128, 3], f32, kind="ExternalInput")
    out_d = nc.dram_tensor("out", [N_SC, SC], f16, kind="ExternalOutput")

    with TileContext(nc) as tc:
        with tc.tile_pool(name="consts", bufs=1) as consts, \
             tc.tile_pool(name="xp", bufs=4) as xp, \
             tc.tile_pool(name="hp", bufs=4) as hp, \
             tc.tile_pool(name="mp", bufs=3) as mp, \
             tc.tile_pool(name="up", bufs=3) as up, \
             tc.tile_pool(name="ph", bufs=2, space="PSUM") as ph, \
             tc.tile_pool(name="pm", bufs=1, space="PSUM") as pm, \
             tc.tile_pool(name="pu", bufs=1, space="PSUM") as pu:

            c16_t = consts.tile([128, N_CONST16], f16)
            nc.sync.dma_start(out=c16_t, in_=c16_d[:, :])
            c8_t = consts.tile([64, 2, 96], f8)
            nc.sync.dma_start(out=c8_t, in_=c8_d[:, :, :])
            c32_t = consts.tile([128, 3], f32)
            nc.sync.dma_start(out=c32_t, in_=c32_d[:, :])

            wfr = c16_t[0:127, 0:97]      # steady fused f+r
            wf_head = c16_t[96:127, 0:91]  # head: f only
            wr_tail = c16_t[0:91, 0:97]    # tail: r only (cols 0..90 zero)
            wm = c16_t[0:91, O_WM:O_WM + 128]
            wuh = c16_t[0:91, O_WUH:O_WUH + 96]
            wum8 = c8_t[0:64, 0:2, 0:96]
            cm1 = c32_t[0:96, 1:2]

            def body():
                # 5-deep software pipeline: iteration k runs f+r[k] (the r
                # part covering SC k-4), m[k-1], u[k-2] (uM starts the
                # accumulation group) — every cross-engine dependency gets a
                # full iteration of slack, so the steady-state period is the
                # busiest engine, not the layer chain.
                h_ts = {}
                m_ts = {}
                u_ts = {}

                # head x tiles for SCs 0-3 (no u tile exists yet to carry x)
                xh_ts = {}
                for j in range(min(r_lag, N_SC)):
                    x_t = xp.tile([128, SC], f16, tag="xh", name="xh_t")
                    nc.sync.dma_start(out=x_t[96:128, :], in_=x_d[j])
                    xh_ts[j] = x_t

                for k in range(N_SC + r_lag):
                    # ---- stage f(+r): superchunk k (r part: SC k-4) ----
                    psum_h = ph.tile([97, SC], f32, tag="psh")
                    if k < r_lag:
                        x_t = xh_ts.pop(k)
                        for s2 in range(2):
                            sl = slice(512 * s2, 512 * (s2 + 1))
                            nc.tensor.matmul(out=psum_h[0:91, sl], lhsT=wf_head,
                                             rhs=x_t[96:127, sl], start=True,
                                             stop=True, tile_position=(96, 0))
                    elif k < N_SC:
                        xu_t = u_ts.pop(k - r_lag)
                        for s2 in range(2):
                            sl = slice(512 * s2, 512 * (s2 + 1))
                            nc.tensor.matmul(out=psum_h[0:97, sl], lhsT=wfr,
                                             rhs=xu_t[0:127, sl], start=True,
                                             stop=True)
                    else:
                        xu_t = u_ts.pop(k - r_lag)
                        for s2 in range(2):
                            sl = slice(512 * s2, 512 * (s2 + 1))
                            nc.tensor.matmul(out=psum_h[0:97, sl], lhsT=wr_tail,
                                             rhs=xu_t[0:91, sl], start=True,
                                             stop=True)

                    # ---- h drain: tanh(psum_h) -> h_t; covers row 96 so
                    # tanh(r[k-4]) rides for free (host inverts it) ----
                    if k < N_SC:
                        rows = 97 if k >= r_lag else D91
                        h_t = hp.tile([97, SC], f16, tag="h", name="h_t")
                        nc.scalar.activation(out=h_t[0:rows, 0:A],
                                             in_=psum_h[0:rows, 0:A], func=Tanh)
                        nc.vector._custom_dve(tanh7, out=h_t[0:rows, A:SC],
                                              in0=psum_h[0:rows, A:SC],
                                              in1=c32_t[0:rows, 0:1],
                                              s0=CH[1], s1=CH[2],
                                              imm2=CH[3])
                        h_ts[k] = h_t
                    else:
                        # tail: same drain shape as steady state (rows 0-90 of
                        # psum are computed zeros; only row 96 is live)
                        h_t = hp.tile([97, SC], f16, tag="h", name="h_t")
                        nc.scalar.activation(out=h_t[0:97, 0:A],
                                             in_=psum_h[0:97, 0:A], func=Tanh)
                        nc.vector._custom_dve(tanh7, out=h_t[0:97, A:SC],
                                              in0=psum_h[0:97, A:SC],
                                              in1=c32_t[0:97, 0:1],
                                              s0=CH[1], s1=CH[2],
                                              imm2=CH[3])

                    # ---- r store: DMA tanh(r[k-4]) row to DRAM ----
                    if k >= r_lag:
                        kr = k - r_lag
                        nc.sync.dma_start(out=out_d[kr:kr + 1, :],
                                          in_=h_t[96:97, :])

                    # ---- stage m: superchunk k-1; psum_m is 128 rows (90
                    # real + 38 computed zeros) so the fp8 k-tile split
                    # [0:64] / [64:128] is fully initialized; drains write
                    # the two k-tiles of the interleaved fp8 m tile ----
                    if 1 <= k < N_SC + 1:
                        km = k - 1
                        h_t = h_ts[km]
                        psum_m = pm.tile([128, SC], f32, tag="psm")
                        for s2 in range(2):
                            sl = slice(512 * s2, 512 * (s2 + 1))
                            nc.tensor.matmul(out=psum_m[0:128, sl], lhsT=wm,
                                             rhs=h_t[0:D91, sl],
                                             start=True, stop=True)
                        m_t = mp.tile([64, 2, SC], f8, tag="m", name="m_t")
                        nc.scalar.activation(out=m_t[0:64, 0, 0:A],
                                             in_=psum_m[0:64, 0:A], func=Tanh)
                        nc.scalar.activation(out=m_t[0:64, 1, 0:SC],
                                             in_=psum_m[64:128, 0:SC], func=Tanh)
                        nc.vector._custom_dve(tanh7, out=m_t[0:64, 0, A:SC],
                                              in0=psum_m[0:64, A:SC],
                                              in1=c32_t[0:64, 1:2],
                                              s0=CM[1], s1=CM[2], imm2=CM[3])
                        m_ts[km] = m_t

                    # ---- stage u: superchunk k-2 (uM starts, uh stops) ----
                    if 2 <= k < N_SC + 2:
                        ku = k - 2
                        h_t = h_ts.pop(ku)
                        m_t = m_ts.pop(ku)
                        psum_u = pu.tile([96, SC], f32, tag="psu")
                        for s2 in range(2):
                            sl = slice(512 * s2, 512 * (s2 + 1))
                            nc.tensor.matmul(out=psum_u[0:96, sl], lhsT=wum8,
                                             rhs=m_t[0:64, 0:2, sl],
                                             start=True, stop=False,
                                             perf_mode=DR)
                            nc.tensor.matmul(out=psum_u[0:96, sl], lhsT=wuh,
                                             rhs=h_t[0:D91, sl],
                                             start=False, stop=True)
                        u_t = up.tile([128, SC], f16, tag="u", name="u_t")
                        nc.scalar.activation(out=u_t[0:96, 0:A],
                                             in_=psum_u[0:96, 0:A], func=Tanh)
                        nc.vector._custom_dve(tanh7, out=u_t[0:96, A:SC],
                                              in0=psum_u[0:96, A:SC],
                                              in1=cm1, s0=CM[1], s1=CM[2],
                                              imm2=CM[3])
                        u_ts[ku] = u_t
                        # x for SC ku+4 rides rows 96-127 of this u tile
                        if ku + r_lag < N_SC:
                            nc.sync.dma_start(out=u_t[96:128, :],
                                              in_=x_d[ku + r_lag])

            if reps == 1:
                body()
            else:
                with tc.For_i(0, reps, staggered_reset=True):
                    body()

    _lower_custom_dve(nc, mybir)
    _split_multi_waits(nc, mybir)
    return nc


def _invert_r(t16, A=None):
    """Invert the drain nonlinearity on the readout row: columns [0:A] went
    through the ACT table tanh (invert with arctanh), columns [A:SC] through
    the degree-7 DVE poly (invert with Newton; arctanh is a ~2e-3 initial
    guess so 3 iterations reach fp32 roundoff)."""
    if A is None:
        A = ACT_COLS
    t = np.asarray(t16, np.float32)
    r = np.empty_like(t)
    lim = np.float32(0.9999997)
    r[:, :A] = np.arctanh(np.clip(t[:, :A], -lim, lim))
    td = t[:, A:]
    y = np.arctanh(np.clip(td, -lim, lim))
    c0, c1, c2, c3 = [np.float32(c) for c in CH]
    three, five, seven = np.float32(3), np.float32(5), np.float32(7)
    for _ in range(3):
        s = y * y
        f = y * (((c3 * s + c2) * s + c1) * s + c0) - td
        df = ((seven * c3 * s + five * c2) * s + three * c1) * s + c0
        y = y - f / df
    r[:, A:] = y
    return r


def _make_in_map(inputs, xc):
    c16, c32, c8 = _build_weight_blocks(
        inputs["Wf"], inputs["bf"], inputs["Wm"], inputs["bm"],
        inputs["Wu"], inputs["bu"], inputs["Wr"], inputs["br"],
    )
    return {"x_prep": _prep_core_x(xc), "consts16": c16, "consts32": c32,
            "consts8": c8}


def _run(inputs, trace=False):
    from concourse.bass_utils import run_bass_kernel_spmd

    x = np.asarray(inputs["x"], np.float32).reshape(B, NODES * F_IN)
    c16, c32, c8 = _build_weight_blocks(
        inputs["Wf"], inputs["bf"], inputs["Wm"], inputs["bm"],
        inputs["Wu"], inputs["bu"], inputs["Wr"], inputs["br"],
    )

    nc = _build_program()

    in_maps = []
    for c in range(N_CORES):
        xc = x[c * B_CORE:(c + 1) * B_CORE]
        in_maps.append({
            "x_prep": _prep_core_x(xc),
            "consts16": c16,
            "consts32": c32,
            "consts8": c8,
        })

    res = run_bass_kernel_spmd(nc, in_maps, list(range(N_CORES)), trace=trace)
    outs = [_invert_r(res.results[c]["out"]).reshape(B_CORE)
            for c in range(N_CORES)]
    full = np.concatenate(outs).reshape(B, 1).astype(np.float32)
    return full, res


def kernel(**inputs):
    full, _ = _run(inputs, trace=False)
    return full


# revision 19
# speedup vs baseline: 1.3065x; 1.3065x over previous
"""Trainium2 Bass kernel for nn_MessagePassing (ring GNN, 5 nodes, 18 hid).

Math (per batch element b, node n, ring of 5):
  h_n = tanh(x_n @ Wf + bf)                       x_n in R^6, h_n in R^18
  M_n = tanh(h_n @ Wm[:18] + h_{n+1} @ Wm[18:] + bm)
  U_n = tanh(M_{n-1} @ Wu[:18] + h_n @ (Wu[18:36]+Wu[36:54]) + bu)
  out = concat(U_0..U_4) @ Wr + br

Layout: feature-major — each batch element is one 90-row column
(5 nodes x 18 hid); batch runs along the free axis, 1024 columns per
superchunk (SC).  All weights are fp16; activations are fp16 in SBUF.

v1 change vs baseline: the readout matmul is FUSED into the f matmul.
The stationary weight block for the f stage carries one extra output
column (91) holding Wr over contraction rows 0-90 (the u values of SC
k-4, which live in rows 0-90 of the same rhs tile that carries x for
SC k in rows 96-126).  This removes the separate readout stream: PE
work drops from 5 to 4 1024-column streams per SC.  psum_h row 96 then
holds r[k-4] and is drained by a DVE row-copy into the store tile.

Engine plan per SC:
  PE    : f+r (rows 0-126 -> out 0-91), m (0-90 -> 0-89),
          uM+uh (accumulating pair -> out 0-95)
  ACT   : exact-table tanh of psum columns [0:ACT_COLS] for h/m/u
  DVE   : TANH7_ANT poly drain of columns [ACT_COLS:1024] + the
          [1,1024] r-row copy psum_h[91] -> store tile
  SP DMA: x in (1 SC per load, into rows 96-127 of the u tile of SC
          k-4), out (1 store per 8 SCs)

The loop runs N_SC+4 iterations: iterations 0-3 are f-only (no u yet;
separate head x tiles), iterations N_SC..N_SC+3 are r-only tail
matmuls (lhsT rows 0-90, so the x rows are never read).  The u drain
writes zeros into rows 91-95 (the wuh/wum blocks carry 5 zero output
columns) so the fused matmul never reads uninitialized partitions.

Biases ride the matmuls exactly as in the baseline: x carries a
ones-row (tile row 126) whose weight row holds bf; h and u carry a
carrier row (row 90, value tanh(CARRIER)) whose weight entries are
biases scaled by 1/tanh(CARRIER); br rides the fused readout column at
the u-carrier row.
"""

import os
import sys

import numpy as np

if not any(os.path.isdir(os.path.join(p, "concourse")) for p in sys.path if p):
    sys.path.insert(0, "/opt/trn_rl_repo")

N_HID = 18
NODES = 5
F_IN = 6
B = 262144
N_CORES = 8
B_CORE = B // N_CORES   # 32768
SC = 1024               # batch columns per superchunk
N_SC = B_CORE // SC     # 32
D90 = NODES * N_HID     # 90
D91 = D90 + 1

ACT_COLS = 537          # ACT's tanh share per layer; DVE takes the rest
CARRIER = 1.2           # bias carrier pre-activation on h/u row 90

# degree-7 odd minimax tanh coefficients: x*(c0 + c1 s + c2 s^2 + c3 s^3)
CH = (0.99191345085891702, -0.29147337765012465,
      0.069278752318150422, -0.0073386055200310675)   # fit on [0, 1.8]
CM = (0.99880200804513142, -0.32174878994875622,
      0.10166490186270968, -0.017242076990493044)     # fit on [0, 1.25]

_TANH7 = None


def _poly_tanh_np(v, c):
    v = np.asarray(v, np.float32)
    s = v * v
    return v * (((np.float32(c[3]) * s + np.float32(c[2])) * s
                 + np.float32(c[1])) * s + np.float32(c[0]))


def _register_tanh7():
    """Register the TANH7_ANT custom DVE op (idempotent)."""
    global _TANH7
    if _TANH7 is not None:
        return _TANH7
    import concourse.dve_ops as dve_ops
    from concourse.dve_ops import DveOp, OPS, CUSTOM_DVE_SPECS, _SUB_OPCODE_FOR_NAME
    from concourse.dve_spec import Spec, Src0, C0, C1, C2, C3, lower, _spill_c3_to_src1
    from concourse.dve_uop import DveOpSpec

    name = "TANH7_ANT"
    for op in OPS:
        if op.name == name:
            _TANH7 = op
            return op

    # y = x * (((C2*s + C1)*s + C0)*s + C3),  s = x^2   [8 ALU ops exactly]
    # C0=c1 (s0), C1=c2 (s1), C2=c3 (imm2), C3=c0 (spilled via in1)
    s = Src0 * Src0
    body = Src0 * (((C2 * s + C1) * s + C0) * s + C3)
    body = _spill_c3_to_src1(body)

    def _ref(in0, in1, c0, c1, c2):
        in0 = np.asarray(in0, np.float32)
        s = in0 * in0
        c3v = np.asarray(in1, np.float32) if in1 is not None else np.float32(0)
        return in0 * (((np.float32(c2) * s + np.float32(c1)) * s
                       + np.float32(c0)) * s + c3v)

    spec = Spec(body=body, reference=_ref)

    from concourse.dve_ops import get_dve_sub_opcode  # noqa: F401
    row = max(_SUB_OPCODE_FOR_NAME.values()) + 1
    shas = {}
    for ver in ("v3", "v4"):
        tmp = DveOpSpec(name=name, opcode=row, uops=lower(spec, ver=ver),
                       rd1_en=True)
        shas[ver] = tmp.sha(ver)

    op = DveOp(name, spec, subdim=False, uops_sha=shas)
    OPS.append(op)
    CUSTOM_DVE_SPECS[name] = spec
    _SUB_OPCODE_FOR_NAME[name] = row
    _TANH7 = op
    return op


# c16 column map: 0:97 wfr | 97:187 wm | 187:283 wuh | 283:379 wum
# (wfr cols 91-95 are zero pad so the readout column lands on psum row 96,
# the PSUM-quadrant-aligned partition the DVE row-copy is allowed to read)
O_WM = 97
O_WUH = 187
O_WUM = 283
N_CONST16 = 379


def _build_weight_blocks(Wf, bf, Wm, bm, Wu, bu, Wr, br):
    f32 = np.float32
    Wf = np.asarray(Wf, f32); bf = np.asarray(bf, f32)
    Wm = np.asarray(Wm, f32); bm = np.asarray(bm, f32)
    Wu = np.asarray(Wu, f32); bu = np.asarray(bu, f32)
    Wr = np.asarray(Wr, f32); br = np.asarray(br, f32)
    tc15 = float(np.tanh(CARRIER))

    c16 = np.zeros((128, N_CONST16), f32)
    # --- wfr: f rows 96..126 (x features + ones-row 126) -> cols 0..90,
    #     readout rows 0..90 (u of SC k-4) -> col 91 ---
    for n in range(NODES):
        for f in range(F_IN):
            c16[96 + F_IN * n + f, N_HID * n:N_HID * n + N_HID] = Wf[f]
        c16[126, N_HID * n:N_HID * n + N_HID] = bf
    c16[126, 90] = CARRIER  # carrier column -> psum_h row 90 = CARRIER
    c16[0:D90, 96] = Wr.reshape(D90)
    c16[90, 96] = br[0] / tc15  # br rides the u carrier row

    # --- wm: rows 0..90, 90 out cols; output col block n holds M_{n-1} ---
    o = O_WM
    for n in range(NODES):
        e = (n - 1) % NODES     # edge index: M_e = tanh(h_e Wm1 + h_{e+1} Wm2)
        c16[N_HID * e:N_HID * e + N_HID, o + N_HID * n:o + N_HID * n + N_HID] += Wm[:N_HID]
        c16[N_HID * ((e + 1) % NODES):N_HID * ((e + 1) % NODES) + N_HID,
            o + N_HID * n:o + N_HID * n + N_HID] += Wm[N_HID:]
        c16[90, o + N_HID * n:o + N_HID * n + N_HID] = bm / tc15

    # --- wuh: rows 0..90, 96 out cols (91..95 zero so psum_u rows 91..95
    #     are initialized; col 90 = carrier for br) ---
    o = O_WUH
    wuh = Wu[N_HID:2 * N_HID] + Wu[2 * N_HID:3 * N_HID]
    for n in range(NODES):
        c16[N_HID * n:N_HID * n + N_HID, o + N_HID * n:o + N_HID * n + N_HID] = wuh
        c16[90, o + N_HID * n:o + N_HID * n + N_HID] = bu / tc15
    c16[90, o + 90] = CARRIER / tc15  # psum_u row 90 = CARRIER

    # --- wum: rows 0..89 (m_t is pre-rolled), 96 out cols (90..95 zero) ---
    o = O_WUM
    for n in range(NODES):
        c16[N_HID * n:N_HID * n + N_HID, o + N_HID * n:o + N_HID * n + N_HID] = Wu[:N_HID]

    # --- f32 consts: poly c0 per layer + br ---
    c32 = np.zeros((128, 3), f32)
    c32[:, 0] = CH[0]
    c32[:, 1] = CM[0]
    c32[:, 2] = br[0]
    return c16.astype(np.float16), c32


def _prep_core_x(xc):
    """[B_CORE, 30] -> [N_SC, 32, SC] fp16; row 30 = 1.0 (bias row)."""
    arr = np.zeros((N_SC, 32, SC), np.float32)
    arr[:, :30] = xc.reshape(N_SC, SC, 30).transpose(0, 2, 1)
    arr[:, 30] = 1.0
    return np.ascontiguousarray(arr).astype(np.float16)


def _split_multi_waits(nc, mybir):
    """walrus's per-instruction sync-wait encoding holds only one wait per
    opcode struct; hoist extra waits onto same-engine NoOps placed before
    the instruction."""
    n = 0
    for fn in nc.m.functions:
        for bb in fn.blocks:
            new_insts = []
            for inst in bb.instructions:
                si = inst.sync_info
                if si is not None and si.on_wait and len(si.on_wait) > 1:
                    waits = list(si.on_wait)
                    for w in waits[:-1]:
                        n += 1
                        nop = mybir.InstNoOp(name=f"I-waitnop-{n}", ins=[], outs=[])
                        nop.engine = inst.engine
                        nop.sync_info = mybir.SyncInfo(on_wait=[w], on_update=[])
                        nc.register_instruction(nop)
                        new_insts.append(nop)
                    inst.sync_info = mybir.SyncInfo(
                        on_wait=[waits[-1]], on_update=list(si.on_update or [])
                    )
                new_insts.append(inst)
            if n:
                bb.instructions = new_insts
    return n


def _lower_custom_dve(nc, mybir):
    """Pack InstCustomDveAnt into ISA bytes (the walrus flow expects the
    bytes pre-packed; only the Bacc flow runs codegen_inst_isa_subclasses)."""
    for fn in nc.m.functions:
        for bb in fn.blocks:
            new = []
            for inst in bb.instructions:
                if isinstance(inst, mybir.InstCustomDveAnt):
                    new.extend(mybir.codegen_inst_isa_one(inst, nc._state, nc.isa))
                else:
                    new.append(inst)
            bb.instructions = new


def _build_program(reps=1, act_cols=None, r_lag=4):
    import concourse.bass as bass
    import concourse.mybir as mybir
    from concourse.tile import TileContext

    tanh7 = _register_tanh7()

    f32 = mybir.dt.float32
    f16 = mybir.dt.float16
    Tanh = mybir.ActivationFunctionType.Tanh
    A = ACT_COLS if act_cols is None else act_cols

    nc = bass.Bass("TRN2")
    x_d = nc.dram_tensor("x_prep", [N_SC, 32, SC], f16, kind="ExternalInput")
    c16_d = nc.dram_tensor("consts16", [128, N_CONST16], f16, kind="ExternalInput")
    c32_d = nc.dram_tensor("consts32", [128, 3], f32, kind="ExternalInput")
    out_d = nc.dram_tensor("out", [N_SC, SC], f16, kind="ExternalOutput")

    with TileContext(nc) as tc:
        with tc.tile_pool(name="consts", bufs=1) as consts, \
             tc.tile_pool(name="xp", bufs=4) as xp, \
             tc.tile_pool(name="hp", bufs=4) as hp, \
             tc.tile_pool(name="mp", bufs=3) as mp, \
             tc.tile_pool(name="up", bufs=3) as up, \
             tc.tile_pool(name="ph", bufs=2, space="PSUM") as ph, \
             tc.tile_pool(name="pm", bufs=1, space="PSUM") as pm, \
             tc.tile_pool(name="pu", bufs=1, space="PSUM") as pu:

            c16_t = consts.tile([128, N_CONST16], f16)
            nc.sync.dma_start(out=c16_t, in_=c16_d[:, :])
            c32_t = consts.tile([128, 3], f32)
            nc.sync.dma_start(out=c32_t, in_=c32_d[:, :])

            wfr = c16_t[0:127, 0:97]      # steady fused f+r
            wf_head = c16_t[96:127, 0:91]  # head: f only
            wr_tail = c16_t[0:91, 0:97]    # tail: r only (cols 0..90 zero)
            wm = c16_t[0:91, O_WM:O_WM + 90]
            wuh = c16_t[0:91, O_WUH:O_WUH + 96]
            wum = c16_t[0:90, O_WUM:O_WUM + 96]
            cm0 = c32_t[0:90, 1:2]
            cm1 = c32_t[0:96, 1:2]

            def body():
                # 5-deep software pipeline: iteration k runs f+r[k] (the r
                # part covering SC k-4), m[k-1], u[k-2] (uM starts the
                # accumulation group) — every cross-engine dependency gets a
                # full iteration of slack, so the steady-state period is the
                # busiest engine, not the layer chain.
                h_ts = {}
                m_ts = {}
                u_ts = {}

                # head x tiles for SCs 0-3 (no u tile exists yet to carry x)
                xh_ts = {}
                for j in range(min(r_lag, N_SC)):
                    x_t = xp.tile([128, SC], f16, tag="xh", name="xh_t")
                    nc.sync.dma_start(out=x_t[96:128, :], in_=x_d[j])
                    xh_ts[j] = x_t

                for k in range(N_SC + r_lag):
                    # ---- stage f(+r): superchunk k (r part: SC k-4) ----
                    psum_h = ph.tile([97, SC], f32, tag="psh")
                    if k < r_lag:
                        x_t = xh_ts.pop(k)
                        for s2 in range(2):
                            sl = slice(512 * s2, 512 * (s2 + 1))
                            nc.tensor.matmul(out=psum_h[0:91, sl], lhsT=wf_head,
                                             rhs=x_t[96:127, sl], start=True,
                                             stop=True, tile_position=(96, 0))
                    elif k < N_SC:
                        xu_t = u_ts.pop(k - r_lag)
                        for s2 in range(2):
                            sl = slice(512 * s2, 512 * (s2 + 1))
                            nc.tensor.matmul(out=psum_h[0:97, sl], lhsT=wfr,
                                             rhs=xu_t[0:127, sl], start=True,
                                             stop=True)
                    else:
                        xu_t = u_ts.pop(k - r_lag)
                        for s2 in range(2):
                            sl = slice(512 * s2, 512 * (s2 + 1))
                            nc.tensor.matmul(out=psum_h[0:97, sl], lhsT=wr_tail,
                                             rhs=xu_t[0:91, sl], start=True,
                                             stop=True)

                    # ---- h drain: tanh(psum_h) -> h_t; covers row 96 so
                    # tanh(r[k-4]) rides for free (host inverts it) ----
                    if k < N_SC:
                        rows = 97 if k >= r_lag else D91
                        h_t = hp.tile([97, SC], f16, tag="h", name="h_t")
                        nc.scalar.activation(out=h_t[0:rows, 0:A],
                                             in_=psum_h[0:rows, 0:A], func=Tanh)
                        nc.vector._custom_dve(tanh7, out=h_t[0:rows, A:SC],
                                              in0=psum_h[0:rows, A:SC],
                                              in1=c32_t[0:rows, 0:1],
                                              s0=CH[1], s1=CH[2],
                                              imm2=CH[3])
                        h_ts[k] = h_t
                    else:
                        # tail: same drain shape as steady state (rows 0-90 of
                        # psum are computed zeros; only row 96 is live)
                        h_t = hp.tile([97, SC], f16, tag="h", name="h_t")
                        nc.scalar.activation(out=h_t[0:97, 0:A],
                                             in_=psum_h[0:97, 0:A], func=Tanh)
                        nc.vector._custom_dve(tanh7, out=h_t[0:97, A:SC],
                                              in0=psum_h[0:97, A:SC],
                                              in1=c32_t[0:97, 0:1],
                                              s0=CH[1], s1=CH[2],
                                              imm2=CH[3])

                    # ---- r store: DMA tanh(r[k-4]) row to DRAM ----
                    if k >= r_lag:
                        kr = k - r_lag
                        nc.sync.dma_start(out=out_d[kr:kr + 1, :],
                                          in_=h_t[96:97, :])

                    # ---- stage m: superchunk k-1 ----
                    if 1 <= k < N_SC + 1:
                        km = k - 1
                        h_t = h_ts[km]
                        psum_m = pm.tile([D90, SC], f32, tag="psm")
                        for s2 in range(2):
                            sl = slice(512 * s2, 512 * (s2 + 1))
                            nc.tensor.matmul(out=psum_m[0:D90, sl], lhsT=wm,
                                             rhs=h_t[0:D91, sl],
                                             start=True, stop=True)
                        m_t = mp.tile([D90, SC], f16, tag="m", name="m_t")
                        nc.scalar.activation(out=m_t[0:D90, 0:A],
                                             in_=psum_m[0:D90, 0:A], func=Tanh)
                        nc.vector._custom_dve(tanh7, out=m_t[0:D90, A:SC],
                                              in0=psum_m[0:D90, A:SC],
                                              in1=cm0, s0=CM[1], s1=CM[2],
                                              imm2=CM[3])
                        m_ts[km] = m_t

                    # ---- stage u: superchunk k-2 (uM starts, uh stops) ----
                    if 2 <= k < N_SC + 2:
                        ku = k - 2
                        h_t = h_ts.pop(ku)
                        m_t = m_ts.pop(ku)
                        psum_u = pu.tile([96, SC], f32, tag="psu")
                        for s2 in range(2):
                            sl = slice(512 * s2, 512 * (s2 + 1))
                            nc.tensor.matmul(out=psum_u[0:96, sl], lhsT=wum,
                                             rhs=m_t[0:D90, sl],
                                             start=True, stop=False)
                            nc.tensor.matmul(out=psum_u[0:96, sl], lhsT=wuh,
                                             rhs=h_t[0:D91, sl],
                                             start=False, stop=True)
                        u_t = up.tile([128, SC], f16, tag="u", name="u_t")
                        nc.scalar.activation(out=u_t[0:96, 0:A],
                                             in_=psum_u[0:96, 0:A], func=Tanh)
                        nc.vector._custom_dve(tanh7, out=u_t[0:96, A:SC],
                                              in0=psum_u[0:96, A:SC],
                                              in1=cm1, s0=CM[1], s1=CM[2],
                                              imm2=CM[3])
                        u_ts[ku] = u_t
                        # x for SC ku+4 rides rows 96-127 of this u tile
                        if ku + r_lag < N_SC:
                            nc.sync.dma_start(out=u_t[96:128, :],
                                              in_=x_d[ku + r_lag])

            if reps == 1:
                body()
            else:
                with tc.For_i(0, reps, staggered_reset=True):
                    body()

    _lower_custom_dve(nc, mybir)
    _split_multi_waits(nc, mybir)
    return nc


def _invert_r(t16, A=None):
    """Invert the drain nonlinearity on the readout row: columns [0:A] went
    through the ACT table tanh (invert with arctanh), columns [A:SC] through
    the degree-7 DVE poly (invert with Newton; arctanh is a ~2e-3 initial
    guess so 3 iterations reach fp32 roundoff)."""
    if A is None:
        A = ACT_COLS
    t = np.asarray(t16, np.float32)
    r = np.empty_like(t)
    lim = np.float32(0.9999997)
    r[:, :A] = np.arctanh(np.clip(t[:, :A], -lim, lim))
    td = t[:, A:]
    y = np.arctanh(np.clip(td, -lim, lim))
    c0, c1, c2, c3 = [np.float32(c) for c in CH]
    three, five, seven = np.float32(3), np.float32(5), np.float32(7)
    for _ in range(3):
        s = y * y
        f = y * (((c3 * s + c2) * s + c1) * s + c0) - td
        df = ((seven * c3 * s + five * c2) * s + three * c1) * s + c0
        y = y - f / df
    r[:, A:] = y
    return r


def _make_in_map(inputs, xc):
    c16, c32 = _build_weight_blocks(
        inputs["Wf"], inputs["bf"], inputs["Wm"], inputs["bm"],
        inputs["Wu"], inputs["bu"], inputs["Wr"], inputs["br"],
    )
    return {"x_prep": _prep_core_x(xc), "consts16": c16, "consts32": c32}


def _run(inputs, trace=False):
    from concourse.bass_utils import run_bass_kernel_spmd

    x = np.asarray(inputs["x"], np.float32).reshape(B, NODES * F_IN)
    c16, c32 = _build_weight_blocks(
        inputs["Wf"], inputs["bf"], inputs["Wm"], inputs["bm"],
        inputs["Wu"], inputs["bu"], inputs["Wr"], inputs["br"],
    )

    nc = _build_program()

    in_maps = []
    for c in range(N_CORES):
        xc = x[c * B_CORE:(c + 1) * B_CORE]
        in_maps.append({
            "x_prep": _prep_core_x(xc),
            "consts16": c16,
            "consts32": c32,
        })

    res = run_bass_kernel_spmd(nc, in_maps, list(range(N_CORES)), trace=trace)
    outs = [_invert_r(res.results[c]["out"]).reshape(B_CORE)
            for c in range(N_CORES)]
    full = np.concatenate(outs).reshape(B, 1).astype(np.float32)
    return full, res


def kernel(**inputs):
    full, _ = _run(inputs, trace=False)
    return full


# revision 24
# speedup vs baseline: 1.3066x; 1.0001x over previous
"""Trainium2 Bass kernel for nn_MessagePassing (ring GNN, 5 nodes, 18 hid).

Math (per batch element b, node n, ring of 5):
  h_n = tanh(x_n @ Wf + bf)                       x_n in R^6, h_n in R^18
  M_n = tanh(h_n @ Wm[:18] + h_{n+1} @ Wm[18:] + bm)
  U_n = tanh(M_{n-1} @ Wu[:18] + h_n @ (Wu[18:36]+Wu[36:54]) + bu)
  out = concat(U_0..U_4) @ Wr + br

Layout: feature-major — each batch element is one 90-row column
(5 nodes x 18 hid); batch runs along the free axis, 1024 columns per
superchunk (SC).  All weights are fp16; activations are fp16 in SBUF.

Two structural changes vs the 5-stream baseline (164-174us):

1. The readout matmul is FUSED into the f matmul: the f stationary
   block carries one extra output column (96) holding Wr over
   contraction rows 0-90 (the u values of SC k-4, which live in rows
   0-90 of the same rhs tile that carries x for SC k in rows 96-126).
   PE work drops from 5 to 4 1024-column streams per SC.

2. The readout row RIDES THE h TANH DRAIN: the h drain covers psum
   rows 0:97, so row 96 = tanh(r[k-4]) lands in the h tile for free
   (columns already paid for); it is DMA'd out as fp16 and inverted on
   the host (arctanh for ACT columns, exact Newton poly-inverse for
   DVE columns).  No separate readout drain op exists.

Engine plan per SC:
  PE    : f+r (rows 0-126 -> out 0-96), m (0-90 -> 0-89),
          uM+uh (accumulating pair -> out 0-95); 4096 cols total
  ACT   : exact-table tanh of psum columns [0:ACT_COLS] for h/m/u
  DVE   : TANH7_ANT poly drain of columns [ACT_COLS:1024]
  SP DMA: x in (1 SC per load, into rows 96-127 of the u tile of SC
          k-4), tanh(r) row out (1 per SC, 2KB fp16)

Measured on this part (slope method): 113.0us/core = 0.836 ns per PE
column — the PE mid-p-state (1.2 GHz) floor, gapless.  2.4 GHz is
reachable only by >3us gap-free PE streams (microbench confirms
0.4167 ns/col); the drains (real costs: ACT 0.794/col + 350/op, DVE
1.107/col + 176/op) floor at ~1.9-2.3us/SC > the 1.71us/SC PE@2.4
period, and PSUM's 8 banks cannot double-buffer two of the three
psum streams, so sprint phases collapse and the kernel settles at
1.2 GHz.  fp8 DoubleRow on the uM matmul was validated numerically
(1.27e-2) but REGRESSED to 163us on HW (fp16<->fp8-DR mode switching
cost), see kernel_v3 in the transcript.

The loop runs N_SC+4 iterations: iterations 0-3 are f-only (no u yet;
separate head x tiles), iterations N_SC..N_SC+3 are r-only tail
matmuls (lhsT rows 0-90, so the x rows are never read).  The u drain
writes zeros into rows 91-95 (the wuh/wum blocks carry 5 zero output
columns) so the fused matmul never reads uninitialized partitions.

Biases ride the matmuls exactly as in the baseline: x carries a
ones-row (tile row 126) whose weight row holds bf; h and u carry a
carrier row (row 90, value tanh(CARRIER)) whose weight entries are
biases scaled by 1/tanh(CARRIER); br rides the fused readout column at
the u-carrier row.
"""

import os
import sys

import numpy as np

if not any(os.path.isdir(os.path.join(p, "concourse")) for p in sys.path if p):
    sys.path.insert(0, "/opt/trn_rl_repo")

N_HID = 18
NODES = 5
F_IN = 6
B = 262144
N_CORES = 8
B_CORE = B // N_CORES   # 32768
SC = 1024               # batch columns per superchunk
N_SC = B_CORE // SC     # 32
D90 = NODES * N_HID     # 90
D91 = D90 + 1

ACT_COLS = 537          # ACT's tanh share per layer; DVE takes the rest
CARRIER = 1.2           # bias carrier pre-activation on h/u row 90

# degree-7 odd minimax tanh coefficients: x*(c0 + c1 s + c2 s^2 + c3 s^3)
CH = (0.99191345085891702, -0.29147337765012465,
      0.069278752318150422, -0.0073386055200310675)   # fit on [0, 1.8]
CM = (0.99880200804513142, -0.32174878994875622,
      0.10166490186270968, -0.017242076990493044)     # fit on [0, 1.25]

_TANH7 = None


def _poly_tanh_np(v, c):
    v = np.asarray(v, np.float32)
    s = v * v
    return v * (((np.float32(c[3]) * s + np.float32(c[2])) * s
                 + np.float32(c[1])) * s + np.float32(c[0]))


def _register_tanh7():
    """Register the TANH7_ANT custom DVE op (idempotent)."""
    global _TANH7
    if _TANH7 is not None:
        return _TANH7
    import concourse.dve_ops as dve_ops
    from concourse.dve_ops import DveOp, OPS, CUSTOM_DVE_SPECS, _SUB_OPCODE_FOR_NAME
    from concourse.dve_spec import Spec, Src0, C0, C1, C2, C3, lower, _spill_c3_to_src1
    from concourse.dve_uop import DveOpSpec

    name = "TANH7_ANT"
    for op in OPS:
        if op.name == name:
            _TANH7 = op
            return op

    # y = x * (((C2*s + C1)*s + C0)*s + C3),  s = x^2   [8 ALU ops exactly]
    # C0=c1 (s0), C1=c2 (s1), C2=c3 (imm2), C3=c0 (spilled via in1)
    s = Src0 * Src0
    body = Src0 * (((C2 * s + C1) * s + C0) * s + C3)
    body = _spill_c3_to_src1(body)

    def _ref(in0, in1, c0, c1, c2):
        in0 = np.asarray(in0, np.float32)
        s = in0 * in0
        c3v = np.asarray(in1, np.float32) if in1 is not None else np.float32(0)
        return in0 * (((np.float32(c2) * s + np.float32(c1)) * s
                       + np.float32(c0)) * s + c3v)

    spec = Spec(body=body, reference=_ref)

    from concourse.dve_ops import get_dve_sub_opcode  # noqa: F401
    row = max(_SUB_OPCODE_FOR_NAME.values()) + 1
    shas = {}
    for ver in ("v3", "v4"):
        tmp = DveOpSpec(name=name, opcode=row, uops=lower(spec, ver=ver),
                       rd1_en=True)
        shas[ver] = tmp.sha(ver)

    op = DveOp(name, spec, subdim=False, uops_sha=shas)
    OPS.append(op)
    CUSTOM_DVE_SPECS[name] = spec
    _SUB_OPCODE_FOR_NAME[name] = row
    _TANH7 = op
    return op


# c16 column map: 0:97 wfr | 97:187 wm | 187:283 wuh | 283:379 wum
# (wfr cols 91-95 are zero pad so the readout column lands on psum row 96,
# the PSUM-quadrant-aligned partition the DVE row-copy is allowed to read)
O_WM = 97
O_WUH = 187
O_WUM = 283
N_CONST16 = 379


def _build_weight_blocks(Wf, bf, Wm, bm, Wu, bu, Wr, br):
    f32 = np.float32
    Wf = np.asarray(Wf, f32); bf = np.asarray(bf, f32)
    Wm = np.asarray(Wm, f32); bm = np.asarray(bm, f32)
    Wu = np.asarray(Wu, f32); bu = np.asarray(bu, f32)
    Wr = np.asarray(Wr, f32); br = np.asarray(br, f32)
    tc15 = float(np.tanh(CARRIER))

    c16 = np.zeros((128, N_CONST16), f32)
    # --- wfr: f rows 96..126 (x features + ones-row 126) -> cols 0..90,
    #     readout rows 0..90 (u of SC k-4) -> col 91 ---
    for n in range(NODES):
        for f in range(F_IN):
            c16[96 + F_IN * n + f, N_HID * n:N_HID * n + N_HID] = Wf[f]
        c16[126, N_HID * n:N_HID * n + N_HID] = bf
    c16[126, 90] = CARRIER  # carrier column -> psum_h row 90 = CARRIER
    c16[0:D90, 96] = Wr.reshape(D90)
    c16[90, 96] = br[0] / tc15  # br rides the u carrier row

    # --- wm: rows 0..90, 90 out cols; output col block n holds M_{n-1} ---
    o = O_WM
    for n in range(NODES):
        e = (n - 1) % NODES     # edge index: M_e = tanh(h_e Wm1 + h_{e+1} Wm2)
        c16[N_HID * e:N_HID * e + N_HID, o + N_HID * n:o + N_HID * n + N_HID] += Wm[:N_HID]
        c16[N_HID * ((e + 1) % NODES):N_HID * ((e + 1) % NODES) + N_HID,
            o + N_HID * n:o + N_HID * n + N_HID] += Wm[N_HID:]
        c16[90, o + N_HID * n:o + N_HID * n + N_HID] = bm / tc15

    # --- wuh: rows 0..90, 96 out cols (91..95 zero so psum_u rows 91..95
    #     are initialized; col 90 = carrier for br) ---
    o = O_WUH
    wuh = Wu[N_HID:2 * N_HID] + Wu[2 * N_HID:3 * N_HID]
    for n in range(NODES):
        c16[N_HID * n:N_HID * n + N_HID, o + N_HID * n:o + N_HID * n + N_HID] = wuh
        c16[90, o + N_HID * n:o + N_HID * n + N_HID] = bu / tc15
    c16[90, o + 90] = CARRIER / tc15  # psum_u row 90 = CARRIER

    # --- wum: rows 0..89 (m_t is pre-rolled), 96 out cols (90..95 zero) ---
    o = O_WUM
    for n in range(NODES):
        c16[N_HID * n:N_HID * n + N_HID, o + N_HID * n:o + N_HID * n + N_HID] = Wu[:N_HID]

    # --- f32 consts: poly c0 per layer + br ---
    c32 = np.zeros((128, 3), f32)
    c32[:, 0] = CH[0]
    c32[:, 1] = CM[0]
    c32[:, 2] = br[0]
    return c16.astype(np.float16), c32


def _prep_core_x(xc):
    """[B_CORE, 30] -> [N_SC, 32, SC] fp16; row 30 = 1.0 (bias row)."""
    arr = np.zeros((N_SC, 32, SC), np.float32)
    arr[:, :30] = xc.reshape(N_SC, SC, 30).transpose(0, 2, 1)
    arr[:, 30] = 1.0
    return np.ascontiguousarray(arr).astype(np.float16)


def _split_multi_waits(nc, mybir, spread=True, lookback=10):
    """walrus's per-instruction sync-wait encoding holds only one wait per
    opcode struct.  Extra waits are spread onto PRECEDING same-engine
    instructions with a free wait slot (engine queues are in-order, so an
    earlier wait is a strictly stronger ordering -> always correct; it can
    only cost time if the wait is not yet satisfied there, and every wait
    in this kernel has >=1 pipeline iteration of slack).  This keeps the
    PE instruction stream free of NoOps, which break the continuous-
    matmul-run heuristic the p-state ramp needs.  Falls back to same-
    engine NoOps when no slot is found within `lookback` instructions."""
    n_nop = 0
    n_moved = 0
    donor_ok = ("InstMatmult",)
    for fn in nc.m.functions:
        for bb in fn.blocks:
            new_insts = []
            for inst in bb.instructions:
                si = inst.sync_info
                if si is not None and si.on_wait and len(si.on_wait) > 1:
                    extra = list(si.on_wait[:-1])
                    if spread and str(inst.engine).endswith("PE"):
                        for j in range(len(new_insts) - 1,
                                       max(len(new_insts) - 1 - lookback, -1),
                                       -1):
                            if not extra:
                                break
                            prev = new_insts[j]
                            if prev.engine != inst.engine:
                                continue
                            if type(prev).__name__ not in donor_ok:
                                continue
                            if not getattr(prev, "start_tensor_calc", False):
                                # waits on mid-accumulation-group matmuls
                                # break walrus codegen / the HW sequencer
                                continue
                            psi = prev.sync_info
                            if psi is None:
                                prev.sync_info = mybir.SyncInfo(
                                    on_wait=[extra.pop()], on_update=[])
                                n_moved += 1
                            elif not psi.on_wait:
                                prev.sync_info = mybir.SyncInfo(
                                    on_wait=[extra.pop()],
                                    on_update=list(psi.on_update or []))
                                n_moved += 1
                    for w in extra:
                        n_nop += 1
                        nop = mybir.InstNoOp(name=f"I-waitnop-{n_nop}",
                                             ins=[], outs=[])
                        nop.engine = inst.engine
                        nop.sync_info = mybir.SyncInfo(on_wait=[w], on_update=[])
                        nc.register_instruction(nop)
                        new_insts.append(nop)
                    inst.sync_info = mybir.SyncInfo(
                        on_wait=[si.on_wait[-1]],
                        on_update=list(si.on_update or [])
                    )
                new_insts.append(inst)
            bb.instructions = new_insts
    return n_moved, n_nop


def _lower_custom_dve(nc, mybir):
    """Pack InstCustomDveAnt into ISA bytes (the walrus flow expects the
    bytes pre-packed; only the Bacc flow runs codegen_inst_isa_subclasses)."""
    for fn in nc.m.functions:
        for bb in fn.blocks:
            new = []
            for inst in bb.instructions:
                if isinstance(inst, mybir.InstCustomDveAnt):
                    new.extend(mybir.codegen_inst_isa_one(inst, nc._state, nc.isa))
                else:
                    new.append(inst)
            bb.instructions = new


def _build_program(reps=1, act_cols=None, r_lag=4):
    import concourse.bass as bass
    import concourse.mybir as mybir
    from concourse.tile import TileContext

    tanh7 = _register_tanh7()

    f32 = mybir.dt.float32
    f16 = mybir.dt.float16
    Tanh = mybir.ActivationFunctionType.Tanh
    A = ACT_COLS if act_cols is None else act_cols

    nc = bass.Bass("TRN2")
    x_d = nc.dram_tensor("x_prep", [N_SC, 32, SC], f16, kind="ExternalInput")
    c16_d = nc.dram_tensor("consts16", [128, N_CONST16], f16, kind="ExternalInput")
    c32_d = nc.dram_tensor("consts32", [128, 3], f32, kind="ExternalInput")
    out_d = nc.dram_tensor("out", [N_SC, SC], f16, kind="ExternalOutput")

    with TileContext(nc) as tc:
        with tc.tile_pool(name="consts", bufs=1) as consts, \
             tc.tile_pool(name="xp", bufs=4) as xp, \
             tc.tile_pool(name="hp", bufs=4) as hp, \
             tc.tile_pool(name="mp", bufs=3) as mp, \
             tc.tile_pool(name="up", bufs=3) as up, \
             tc.tile_pool(name="ph", bufs=2, space="PSUM") as ph, \
             tc.tile_pool(name="pm", bufs=1, space="PSUM") as pm, \
             tc.tile_pool(name="pu", bufs=1, space="PSUM") as pu:

            c16_t = consts.tile([128, N_CONST16], f16)
            nc.sync.dma_start(out=c16_t, in_=c16_d[:, :])
            c32_t = consts.tile([128, 3], f32)
            nc.sync.dma_start(out=c32_t, in_=c32_d[:, :])

            wfr = c16_t[0:127, 0:97]      # steady fused f+r
            wf_head = c16_t[96:127, 0:91]  # head: f only
            wr_tail = c16_t[0:91, 0:97]    # tail: r only (cols 0..90 zero)
            wm = c16_t[0:91, O_WM:O_WM + 90]
            wuh = c16_t[0:91, O_WUH:O_WUH + 96]
            wum = c16_t[0:90, O_WUM:O_WUM + 96]
            cm0 = c32_t[0:90, 1:2]
            cm1 = c32_t[0:96, 1:2]

            def body():
                # 5-deep software pipeline: iteration k runs f+r[k] (the r
                # part covering SC k-4), m[k-1], u[k-2] (uM starts the
                # accumulation group) — every cross-engine dependency gets a
                # full iteration of slack, so the steady-state period is the
                # busiest engine, not the layer chain.
                h_ts = {}
                m_ts = {}
                u_ts = {}

                # head x tiles for SCs 0-3 (no u tile exists yet to carry x)
                xh_ts = {}
                for j in range(min(r_lag, N_SC)):
                    x_t = xp.tile([128, SC], f16, tag="xh", name="xh_t")
                    nc.sync.dma_start(out=x_t[96:128, :], in_=x_d[j])
                    xh_ts[j] = x_t

                for k in range(N_SC + r_lag):
                    # ---- stage f(+r): superchunk k (r part: SC k-4) ----
                    psum_h = ph.tile([97, SC], f32, tag="psh")
                    if k < r_lag:
                        x_t = xh_ts.pop(k)
                        for s2 in range(2):
                            sl = slice(512 * s2, 512 * (s2 + 1))
                            nc.tensor.matmul(out=psum_h[0:91, sl], lhsT=wf_head,
                                             rhs=x_t[96:127, sl], start=True,
                                             stop=True, tile_position=(96, 0))
                    elif k < N_SC:
                        xu_t = u_ts.pop(k - r_lag)
                        for s2 in range(2):
                            sl = slice(512 * s2, 512 * (s2 + 1))
                            nc.tensor.matmul(out=psum_h[0:97, sl], lhsT=wfr,
                                             rhs=xu_t[0:127, sl], start=True,
                                             stop=True)
                    else:
                        xu_t = u_ts.pop(k - r_lag)
                        for s2 in range(2):
                            sl = slice(512 * s2, 512 * (s2 + 1))
                            nc.tensor.matmul(out=psum_h[0:97, sl], lhsT=wr_tail,
                                             rhs=xu_t[0:91, sl], start=True,
                                             stop=True)

                    # ---- h drain: tanh(psum_h) -> h_t; covers row 96 so
                    # tanh(r[k-4]) rides for free (host inverts it) ----
                    if k < N_SC:
                        rows = 97 if k >= r_lag else D91
                        h_t = hp.tile([97, SC], f16, tag="h", name="h_t")
                        nc.scalar.activation(out=h_t[0:rows, 0:A],
                                             in_=psum_h[0:rows, 0:A], func=Tanh)
                        nc.vector._custom_dve(tanh7, out=h_t[0:rows, A:SC],
                                              in0=psum_h[0:rows, A:SC],
                                              in1=c32_t[0:rows, 0:1],
                                              s0=CH[1], s1=CH[2],
                                              imm2=CH[3])
                        h_ts[k] = h_t
                    else:
                        # tail: same drain shape as steady state (rows 0-90 of
                        # psum are computed zeros; only row 96 is live)
                        h_t = hp.tile([97, SC], f16, tag="h", name="h_t")
                        nc.scalar.activation(out=h_t[0:97, 0:A],
                                             in_=psum_h[0:97, 0:A], func=Tanh)
                        nc.vector._custom_dve(tanh7, out=h_t[0:97, A:SC],
                                              in0=psum_h[0:97, A:SC],
                                              in1=c32_t[0:97, 0:1],
                                              s0=CH[1], s1=CH[2],
                                              imm2=CH[3])

                    # ---- r store: DMA tanh(r[k-4]) row to DRAM ----
                    if k >= r_lag:
                        kr = k - r_lag
                        nc.sync.dma_start(out=out_d[kr:kr + 1, :],
                                          in_=h_t[96:97, :])

                    # ---- stage m: superchunk k-1 ----
                    if 1 <= k < N_SC + 1:
                        km = k - 1
                        h_t = h_ts[km]
                        psum_m = pm.tile([D90, SC], f32, tag="psm")
                        for s2 in range(2):
                            sl = slice(512 * s2, 512 * (s2 + 1))
                            nc.tensor.matmul(out=psum_m[0:D90, sl], lhsT=wm,
                                             rhs=h_t[0:D91, sl],
                                             start=True, stop=True)
                        m_t = mp.tile([D90, SC], f16, tag="m", name="m_t")
                        nc.scalar.activation(out=m_t[0:D90, 0:A],
                                             in_=psum_m[0:D90, 0:A], func=Tanh)
                        nc.vector._custom_dve(tanh7, out=m_t[0:D90, A:SC],
                                              in0=psum_m[0:D90, A:SC],
                                              in1=cm0, s0=CM[1], s1=CM[2],
                                              imm2=CM[3])
                        m_ts[km] = m_t

                    # ---- stage u: superchunk k-2 (uM starts, uh stops) ----
                    if 2 <= k < N_SC + 2:
                        ku = k - 2
                        h_t = h_ts.pop(ku)
                        m_t = m_ts.pop(ku)
                        psum_u = pu.tile([96, SC], f32, tag="psu")
                        for s2 in range(2):
                            sl = slice(512 * s2, 512 * (s2 + 1))
                            nc.tensor.matmul(out=psum_u[0:96, sl], lhsT=wum,
                                             rhs=m_t[0:D90, sl],
                                             start=True, stop=False)
                            nc.tensor.matmul(out=psum_u[0:96, sl], lhsT=wuh,
                                             rhs=h_t[0:D91, sl],
                                             start=False, stop=True)
                        u_t = up.tile([128, SC], f16, tag="u", name="u_t")
                        nc.scalar.activation(out=u_t[0:96, 0:A],
                                             in_=psum_u[0:96, 0:A], func=Tanh)
                        nc.vector._custom_dve(tanh7, out=u_t[0:96, A:SC],
                                              in0=psum_u[0:96, A:SC],
                                              in1=cm1, s0=CM[1], s1=CM[2],
                                              imm2=CM[3])
                        u_ts[ku] = u_t
                        # x for SC ku+4 rides rows 96-127 of this u tile
                        if ku + r_lag < N_SC:
                            nc.sync.dma_start(out=u_t[96:128, :],
                                              in_=x_d[ku + r_lag])

            if reps == 1:
                body()
            else:
                with tc.For_i(0, reps, staggered_reset=True):
                    body()

    _lower_custom_dve(nc, mybir)
    _split_multi_waits(nc, mybir)
    return nc


def _invert_r(t16, A=None):
    """Invert the drain nonlinearity on the readout row: columns [0:A] went
    through the ACT table tanh (invert with arctanh), columns [A:SC] through
    the degree-7 DVE poly (invert with Newton; arctanh is a ~2e-3 initial
    guess so 3 iterations reach fp32 roundoff)."""
    if A is None:
        A = ACT_COLS
    t = np.asarray(t16, np.float32)
    r = np.empty_like(t)
    lim = np.float32(0.9999997)
    r[:, :A] = np.arctanh(np.clip(t[:, :A], -lim, lim))
    td = t[:, A:]
    y = np.arctanh(np.clip(td, -lim, lim))
    c0, c1, c2, c3 = [np.float32(c) for c in CH]
    three, five, seven = np.float32(3), np.float32(5), np.float32(7)
    for _ in range(3):
        s = y * y
        f = y * (((c3 * s + c2) * s + c1) * s + c0) - td
        df = ((seven * c3 * s + five * c2) * s + three * c1) * s + c0
        y = y - f / df
    r[:, A:] = y
    return r


def _make_in_map(inputs, xc):
    c16, c32 = _build_weight_blocks(
        inputs["Wf"], inputs["bf"], inputs["Wm"], inputs["bm"],
        inputs["Wu"], inputs["bu"], inputs["Wr"], inputs["br"],
    )
    return {"x_prep": _prep_core_x(xc), "consts16": c16, "consts32": c32}


def _run(inputs, trace=False):
    from concourse.bass_utils import run_bass_kernel_spmd

    x = np.asarray(inputs["x"], np.float32).reshape(B, NODES * F_IN)
    c16, c32 = _build_weight_blocks(
        inputs["Wf"], inputs["bf"], inputs["Wm"], inputs["bm"],
        inputs["Wu"], inputs["bu"], inputs["Wr"], inputs["br"],
    )

    nc = _build_program()

    in_maps = []
    for c in range(N_CORES):
        xc = x[c * B_CORE:(c + 1) * B_CORE]
        in_maps.append({
            "x_prep": _prep_core_x(xc),
            "consts16": c16,
            "consts32": c32,
        })

    res = run_bass_kernel_spmd(nc, in_maps, list(range(N_CORES)), trace=trace)
    outs = [_invert_r(res.results[c]["out"]).reshape(B_CORE)
            for c in range(N_CORES)]
    full = np.concatenate(outs).reshape(B, 1).astype(np.float32)
    return full, res


def kernel(**inputs):
    full, _ = _run(inputs, trace=False)
    return full


# revision 25
# speedup vs baseline: 1.3475x; 1.0313x over previous
"""Trainium2 Bass kernel for nn_MessagePassing (ring GNN, 5 nodes, 18 hid).

Math (per batch element b, node n, ring of 5):
  h_n = tanh(x_n @ Wf + bf)                       x_n in R^6, h_n in R^18
  M_n = tanh(h_n @ Wm[:18] + h_{n+1} @ Wm[18:] + bm)
  U_n = tanh(M_{n-1} @ Wu[:18] + h_n @ (Wu[18:36]+Wu[36:54]) + bu)
  out = concat(U_0..U_4) @ Wr + br

Layout: feature-major — each batch element is one 90-row column
(5 nodes x 18 hid); batch runs along the free axis, 1024 columns per
superchunk (SC).  All weights are fp16; activations are fp16 in SBUF.

Two structural changes vs the 5-stream baseline (164-174us):

1. The readout matmul is FUSED into the f matmul: the f stationary
   block carries one extra output column (96) holding Wr over
   contraction rows 0-90 (the u values of SC k-4, which live in rows
   0-90 of the same rhs tile that carries x for SC k in rows 96-126).
   PE work drops from 5 to 4 1024-column streams per SC.

2. The readout row RIDES THE h TANH DRAIN: the h drain covers psum
   rows 0:97, so row 96 = tanh(r[k-4]) lands in the h tile for free
   (columns already paid for); it is DMA'd out as fp16 and inverted on
   the host (arctanh for ACT columns, exact Newton poly-inverse for
   DVE columns).  No separate readout drain op exists.

Engine plan per SC:
  PE    : f+r (rows 0-126 -> out 0-96), m (0-90 -> 0-89),
          uM+uh (accumulating pair -> out 0-95); 4096 cols total
  ACT   : exact-table tanh of psum columns [0:ACT_COLS] for h/m/u
  DVE   : TANH7_ANT poly drain of columns [ACT_COLS:1024]
  SP DMA: x in (1 SC per load, into rows 96-127 of the u tile of SC
          k-4), tanh(r) row out (1 per SC, 2KB fp16)

Measured on this part (slope method): 113.0us/core = 0.836 ns per PE
column — the PE mid-p-state (1.2 GHz) floor, gapless.  2.4 GHz is
reachable only by >3us gap-free PE streams (microbench confirms
0.4167 ns/col); the drains (real costs: ACT 0.794/col + 350/op, DVE
1.107/col + 176/op) floor at ~1.9-2.3us/SC > the 1.71us/SC PE@2.4
period, and PSUM's 8 banks cannot double-buffer two of the three
psum streams, so sprint phases collapse and the kernel settles at
1.2 GHz.  fp8 DoubleRow on the uM matmul was validated numerically
(1.27e-2) but REGRESSED to 163us on HW (fp16<->fp8-DR mode switching
cost), see kernel_v3 in the transcript.

The loop runs N_SC+4 iterations: iterations 0-3 are f-only (no u yet;
separate head x tiles), iterations N_SC..N_SC+3 are r-only tail
matmuls (lhsT rows 0-90, so the x rows are never read).  The u drain
writes zeros into rows 91-95 (the wuh/wum blocks carry 5 zero output
columns) so the fused matmul never reads uninitialized partitions.

Biases ride the matmuls exactly as in the baseline: x carries a
ones-row (tile row 126) whose weight row holds bf; h and u carry a
carrier row (row 90, value tanh(CARRIER)) whose weight entries are
biases scaled by 1/tanh(CARRIER); br rides the fused readout column at
the u-carrier row.
"""

import os
import sys

import numpy as np

if not any(os.path.isdir(os.path.join(p, "concourse")) for p in sys.path if p):
    sys.path.insert(0, "/opt/trn_rl_repo")

N_HID = 18
NODES = 5
F_IN = 6
B = 262144
N_CORES = 8
B_CORE = B // N_CORES   # 32768
SC = 1024               # batch columns per superchunk
N_SC = B_CORE // SC     # 32
D90 = NODES * N_HID     # 90
D91 = D90 + 1

ACT_COLS = 537          # ACT's tanh share per layer; DVE takes the rest
CARRIER = 1.2           # bias carrier pre-activation on h/u row 90

# degree-7 odd minimax tanh coefficients: x*(c0 + c1 s + c2 s^2 + c3 s^3)
CH = (0.99191345085891702, -0.29147337765012465,
      0.069278752318150422, -0.0073386055200310675)   # fit on [0, 1.8]
CM = (0.99880200804513142, -0.32174878994875622,
      0.10166490186270968, -0.017242076990493044)     # fit on [0, 1.25]

_TANH7 = None


def _poly_tanh_np(v, c):
    v = np.asarray(v, np.float32)
    s = v * v
    return v * (((np.float32(c[3]) * s + np.float32(c[2])) * s
                 + np.float32(c[1])) * s + np.float32(c[0]))


def _register_tanh7():
    """Register the TANH7_ANT custom DVE op (idempotent)."""
    global _TANH7
    if _TANH7 is not None:
        return _TANH7
    import concourse.dve_ops as dve_ops
    from concourse.dve_ops import DveOp, OPS, CUSTOM_DVE_SPECS, _SUB_OPCODE_FOR_NAME
    from concourse.dve_spec import Spec, Src0, C0, C1, C2, C3, lower, _spill_c3_to_src1
    from concourse.dve_uop import DveOpSpec

    name = "TANH7_ANT"
    for op in OPS:
        if op.name == name:
            _TANH7 = op
            return op

    # y = x * (((C2*s + C1)*s + C0)*s + C3),  s = x^2   [8 ALU ops exactly]
    # C0=c1 (s0), C1=c2 (s1), C2=c3 (imm2), C3=c0 (spilled via in1)
    s = Src0 * Src0
    body = Src0 * (((C2 * s + C1) * s + C0) * s + C3)
    body = _spill_c3_to_src1(body)

    def _ref(in0, in1, c0, c1, c2):
        in0 = np.asarray(in0, np.float32)
        s = in0 * in0
        c3v = np.asarray(in1, np.float32) if in1 is not None else np.float32(0)
        return in0 * (((np.float32(c2) * s + np.float32(c1)) * s
                       + np.float32(c0)) * s + c3v)

    spec = Spec(body=body, reference=_ref)

    from concourse.dve_ops import get_dve_sub_opcode  # noqa: F401
    row = max(_SUB_OPCODE_FOR_NAME.values()) + 1
    shas = {}
    for ver in ("v3", "v4"):
        tmp = DveOpSpec(name=name, opcode=row, uops=lower(spec, ver=ver),
                       rd1_en=True)
        shas[ver] = tmp.sha(ver)

    op = DveOp(name, spec, subdim=False, uops_sha=shas)
    OPS.append(op)
    CUSTOM_DVE_SPECS[name] = spec
    _SUB_OPCODE_FOR_NAME[name] = row
    _TANH7 = op
    return op


# c16 column map: 0:97 wfr | 97:187 wm | 187:283 wuh | 283:379 wum
# (wfr cols 91-95 are zero pad so the readout column lands on psum row 96,
# the PSUM-quadrant-aligned partition the DVE row-copy is allowed to read)
O_WM = 97
O_WUH = 187
O_WUM = 283
N_CONST16 = 379


def _build_weight_blocks(Wf, bf, Wm, bm, Wu, bu, Wr, br):
    f32 = np.float32
    Wf = np.asarray(Wf, f32); bf = np.asarray(bf, f32)
    Wm = np.asarray(Wm, f32); bm = np.asarray(bm, f32)
    Wu = np.asarray(Wu, f32); bu = np.asarray(bu, f32)
    Wr = np.asarray(Wr, f32); br = np.asarray(br, f32)
    tc15 = float(np.tanh(CARRIER))

    c16 = np.zeros((128, N_CONST16), f32)
    # --- wfr: f rows 96..126 (x features + ones-row 126) -> cols 0..90,
    #     readout rows 0..90 (u of SC k-4) -> col 91 ---
    for n in range(NODES):
        for f in range(F_IN):
            c16[96 + F_IN * n + f, N_HID * n:N_HID * n + N_HID] = Wf[f]
        c16[126, N_HID * n:N_HID * n + N_HID] = bf
    c16[126, 90] = CARRIER  # carrier column -> psum_h row 90 = CARRIER
    c16[0:D90, 96] = Wr.reshape(D90)
    c16[90, 96] = br[0] / tc15  # br rides the u carrier row

    # --- wm: rows 0..90, 90 out cols; output col block n holds M_{n-1} ---
    o = O_WM
    for n in range(NODES):
        e = (n - 1) % NODES     # edge index: M_e = tanh(h_e Wm1 + h_{e+1} Wm2)
        c16[N_HID * e:N_HID * e + N_HID, o + N_HID * n:o + N_HID * n + N_HID] += Wm[:N_HID]
        c16[N_HID * ((e + 1) % NODES):N_HID * ((e + 1) % NODES) + N_HID,
            o + N_HID * n:o + N_HID * n + N_HID] += Wm[N_HID:]
        c16[90, o + N_HID * n:o + N_HID * n + N_HID] = bm / tc15

    # --- wuh: rows 0..90, 96 out cols (91..95 zero so psum_u rows 91..95
    #     are initialized; col 90 = carrier for br) ---
    o = O_WUH
    wuh = Wu[N_HID:2 * N_HID] + Wu[2 * N_HID:3 * N_HID]
    for n in range(NODES):
        c16[N_HID * n:N_HID * n + N_HID, o + N_HID * n:o + N_HID * n + N_HID] = wuh
        c16[90, o + N_HID * n:o + N_HID * n + N_HID] = bu / tc15
    c16[90, o + 90] = CARRIER / tc15  # psum_u row 90 = CARRIER

    # --- wum: rows 0..89 (m_t is pre-rolled), 96 out cols (90..95 zero) ---
    o = O_WUM
    for n in range(NODES):
        c16[N_HID * n:N_HID * n + N_HID, o + N_HID * n:o + N_HID * n + N_HID] = Wu[:N_HID]

    # --- f32 consts: poly c0 per layer + br ---
    c32 = np.zeros((128, 3), f32)
    c32[:, 0] = CH[0]
    c32[:, 1] = CM[0]
    c32[:, 2] = br[0]
    return c16.astype(np.float16), c32


def _prep_core_x(xc):
    """[B_CORE, 30] -> [N_SC, 32, SC] fp16; row 30 = 1.0 (bias row)."""
    arr = np.zeros((N_SC, 32, SC), np.float32)
    arr[:, :30] = xc.reshape(N_SC, SC, 30).transpose(0, 2, 1)
    arr[:, 30] = 1.0
    return np.ascontiguousarray(arr).astype(np.float16)


def _split_multi_waits(nc, mybir, spread=False, lookback=10):
    """walrus's per-instruction sync-wait encoding holds only one wait per
    opcode struct.  Extra waits are spread onto PRECEDING same-engine
    instructions with a free wait slot (engine queues are in-order, so an
    earlier wait is a strictly stronger ordering -> always correct; it can
    only cost time if the wait is not yet satisfied there, and every wait
    in this kernel has >=1 pipeline iteration of slack).  This keeps the
    PE instruction stream free of NoOps, which break the continuous-
    matmul-run heuristic the p-state ramp needs.  Falls back to same-
    engine NoOps when no slot is found within `lookback` instructions."""
    n_nop = 0
    n_moved = 0
    donor_ok = ("InstMatmult",)
    for fn in nc.m.functions:
        for bb in fn.blocks:
            new_insts = []
            for inst in bb.instructions:
                si = inst.sync_info
                if si is not None and si.on_wait and len(si.on_wait) > 1:
                    extra = list(si.on_wait[:-1])
                    if spread and str(inst.engine).endswith("PE"):
                        for j in range(len(new_insts) - 1,
                                       max(len(new_insts) - 1 - lookback, -1),
                                       -1):
                            if not extra:
                                break
                            prev = new_insts[j]
                            if prev.engine != inst.engine:
                                continue
                            if type(prev).__name__ not in donor_ok:
                                continue
                            if not getattr(prev, "start_tensor_calc", False):
                                # waits on mid-accumulation-group matmuls
                                # break walrus codegen / the HW sequencer
                                continue
                            psi = prev.sync_info
                            if psi is None:
                                prev.sync_info = mybir.SyncInfo(
                                    on_wait=[extra.pop()], on_update=[])
                                n_moved += 1
                            elif not psi.on_wait:
                                prev.sync_info = mybir.SyncInfo(
                                    on_wait=[extra.pop()],
                                    on_update=list(psi.on_update or []))
                                n_moved += 1
                    for w in extra:
                        n_nop += 1
                        nop = mybir.InstNoOp(name=f"I-waitnop-{n_nop}",
                                             ins=[], outs=[])
                        nop.engine = inst.engine
                        nop.sync_info = mybir.SyncInfo(on_wait=[w], on_update=[])
                        nc.register_instruction(nop)
                        new_insts.append(nop)
                    inst.sync_info = mybir.SyncInfo(
                        on_wait=[si.on_wait[-1]],
                        on_update=list(si.on_update or [])
                    )
                new_insts.append(inst)
            bb.instructions = new_insts
    return n_moved, n_nop


def _lower_custom_dve(nc, mybir):
    """Pack InstCustomDveAnt into ISA bytes (the walrus flow expects the
    bytes pre-packed; only the Bacc flow runs codegen_inst_isa_subclasses)."""
    for fn in nc.m.functions:
        for bb in fn.blocks:
            new = []
            for inst in bb.instructions:
                if isinstance(inst, mybir.InstCustomDveAnt):
                    new.extend(mybir.codegen_inst_isa_one(inst, nc._state, nc.isa))
                else:
                    new.append(inst)
            bb.instructions = new


def _build_program(reps=1, act_cols=None, r_lag=4):
    import concourse.bass as bass
    import concourse.mybir as mybir
    from concourse.tile import TileContext

    tanh7 = _register_tanh7()

    f32 = mybir.dt.float32
    f16 = mybir.dt.float16
    Tanh = mybir.ActivationFunctionType.Tanh
    A = ACT_COLS if act_cols is None else act_cols

    nc = bass.Bass("TRN2")
    x_d = nc.dram_tensor("x_prep", [N_SC, 32, SC], f16, kind="ExternalInput")
    c16_d = nc.dram_tensor("consts16", [128, N_CONST16], f16, kind="ExternalInput")
    c32_d = nc.dram_tensor("consts32", [128, 3], f32, kind="ExternalInput")
    out_d = nc.dram_tensor("out", [N_SC, SC], f16, kind="ExternalOutput")

    with TileContext(nc) as tc:
        with tc.tile_pool(name="consts", bufs=1) as consts, \
             tc.tile_pool(name="xp", bufs=4) as xp, \
             tc.tile_pool(name="hp", bufs=4) as hp, \
             tc.tile_pool(name="mp", bufs=3) as mp, \
             tc.tile_pool(name="up", bufs=3) as up, \
             tc.tile_pool(name="ph", bufs=2, space="PSUM") as ph, \
             tc.tile_pool(name="pm", bufs=1, space="PSUM") as pm, \
             tc.tile_pool(name="pu", bufs=1, space="PSUM") as pu:

            c16_t = consts.tile([128, N_CONST16], f16)
            nc.sync.dma_start(out=c16_t, in_=c16_d[:, :])
            c32_t = consts.tile([128, 3], f32)
            nc.sync.dma_start(out=c32_t, in_=c32_d[:, :])

            wfr = c16_t[0:127, 0:97]      # steady fused f+r
            wf_head = c16_t[96:127, 0:91]  # head: f only
            wr_tail = c16_t[0:91, 0:97]    # tail: r only (cols 0..90 zero)
            wm = c16_t[0:91, O_WM:O_WM + 90]
            wuh = c16_t[0:91, O_WUH:O_WUH + 96]
            wum = c16_t[0:90, O_WUM:O_WUM + 96]
            cm0 = c32_t[0:90, 1:2]
            cm1 = c32_t[0:96, 1:2]

            def body():
                # 5-deep software pipeline: iteration k runs f+r[k] (the r
                # part covering SC k-4), m[k-1], u[k-2] (uM starts the
                # accumulation group) — every cross-engine dependency gets a
                # full iteration of slack, so the steady-state period is the
                # busiest engine, not the layer chain.
                h_ts = {}
                m_ts = {}
                u_ts = {}

                # head x tiles for SCs 0-3 (no u tile exists yet to carry x)
                xh_ts = {}
                for j in range(min(r_lag, N_SC)):
                    x_t = xp.tile([128, SC], f16, tag="xh", name="xh_t")
                    nc.sync.dma_start(out=x_t[96:128, :], in_=x_d[j])
                    xh_ts[j] = x_t

                for k in range(N_SC + r_lag):
                    # ---- stage f(+r): superchunk k (r part: SC k-4) ----
                    psum_h = ph.tile([97, SC], f32, tag="psh")
                    if k < r_lag:
                        x_t = xh_ts.pop(k)
                        for s2 in range(2):
                            sl = slice(512 * s2, 512 * (s2 + 1))
                            nc.tensor.matmul(out=psum_h[0:91, sl], lhsT=wf_head,
                                             rhs=x_t[96:127, sl], start=True,
                                             stop=True, tile_position=(96, 0))
                    elif k < N_SC:
                        xu_t = u_ts.pop(k - r_lag)
                        for s2 in range(2):
                            sl = slice(512 * s2, 512 * (s2 + 1))
                            nc.tensor.matmul(out=psum_h[0:97, sl], lhsT=wfr,
                                             rhs=xu_t[0:127, sl], start=True,
                                             stop=True)
                    else:
                        xu_t = u_ts.pop(k - r_lag)
                        for s2 in range(2):
                            sl = slice(512 * s2, 512 * (s2 + 1))
                            nc.tensor.matmul(out=psum_h[0:97, sl], lhsT=wr_tail,
                                             rhs=xu_t[0:91, sl], start=True,
                                             stop=True)

                    # ---- h drain: tanh(psum_h) -> h_t; covers row 96 so
                    # tanh(r[k-4]) rides for free (host inverts it) ----
                    if k < N_SC:
                        rows = 97 if k >= r_lag else D91
                        h_t = hp.tile([97, SC], f16, tag="h", name="h_t")
                        nc.scalar.activation(out=h_t[0:rows, 0:A],
                                             in_=psum_h[0:rows, 0:A], func=Tanh)
                        nc.vector._custom_dve(tanh7, out=h_t[0:rows, A:SC],
                                              in0=psum_h[0:rows, A:SC],
                                              in1=c32_t[0:rows, 0:1],
                                              s0=CH[1], s1=CH[2],
                                              imm2=CH[3])
                        h_ts[k] = h_t
                    else:
                        # tail: same drain shape as steady state (rows 0-90 of
                        # psum are computed zeros; only row 96 is live)
                        h_t = hp.tile([97, SC], f16, tag="h", name="h_t")
                        nc.scalar.activation(out=h_t[0:97, 0:A],
                                             in_=psum_h[0:97, 0:A], func=Tanh)
                        nc.vector._custom_dve(tanh7, out=h_t[0:97, A:SC],
                                              in0=psum_h[0:97, A:SC],
                                              in1=c32_t[0:97, 0:1],
                                              s0=CH[1], s1=CH[2],
                                              imm2=CH[3])

                    # ---- r store: DMA tanh(r[k-4]) row to DRAM ----
                    if k >= r_lag:
                        kr = k - r_lag
                        nc.sync.dma_start(out=out_d[kr:kr + 1, :],
                                          in_=h_t[96:97, :])

                    # ---- stage m: superchunk k-1 ----
                    if 1 <= k < N_SC + 1:
                        km = k - 1
                        h_t = h_ts[km]
                        psum_m = pm.tile([D90, SC], f32, tag="psm")
                        for s2 in range(2):
                            sl = slice(512 * s2, 512 * (s2 + 1))
                            nc.tensor.matmul(out=psum_m[0:D90, sl], lhsT=wm,
                                             rhs=h_t[0:D91, sl],
                                             start=True, stop=True)
                        m_t = mp.tile([D90, SC], f16, tag="m", name="m_t")
                        nc.scalar.activation(out=m_t[0:D90, 0:A],
                                             in_=psum_m[0:D90, 0:A], func=Tanh)
                        nc.vector._custom_dve(tanh7, out=m_t[0:D90, A:SC],
                                              in0=psum_m[0:D90, A:SC],
                                              in1=cm0, s0=CM[1], s1=CM[2],
                                              imm2=CM[3])
                        m_ts[km] = m_t

                    # ---- stage u: superchunk k-2 (uM starts, uh stops) ----
                    if 2 <= k < N_SC + 2:
                        ku = k - 2
                        h_t = h_ts.pop(ku)
                        m_t = m_ts.pop(ku)
                        psum_u = pu.tile([96, SC], f32, tag="psu")
                        for s2 in range(2):
                            sl = slice(512 * s2, 512 * (s2 + 1))
                            nc.tensor.matmul(out=psum_u[0:96, sl], lhsT=wum,
                                             rhs=m_t[0:D90, sl],
                                             start=True, stop=False)
                            nc.tensor.matmul(out=psum_u[0:96, sl], lhsT=wuh,
                                             rhs=h_t[0:D91, sl],
                                             start=False, stop=True)
                        u_t = up.tile([128, SC], f16, tag="u", name="u_t")
                        nc.scalar.activation(out=u_t[0:96, 0:A],
                                             in_=psum_u[0:96, 0:A], func=Tanh)
                        nc.vector._custom_dve(tanh7, out=u_t[0:96, A:SC],
                                              in0=psum_u[0:96, A:SC],
                                              in1=cm1, s0=CM[1], s1=CM[2],
                                              imm2=CM[3])
                        u_ts[ku] = u_t
                        # x for SC ku+4 rides rows 96-127 of this u tile
                        if ku + r_lag < N_SC:
                            nc.sync.dma_start(out=u_t[96:128, :],
                                              in_=x_d[ku + r_lag])

            if reps == 1:
                body()
            else:
                with tc.For_i(0, reps, staggered_reset=True):
                    body()

    _lower_custom_dve(nc, mybir)
    _split_multi_waits(nc, mybir)
    return nc


def _invert_r(t16, A=None):
    """Invert the drain nonlinearity on the readout row: columns [0:A] went
    through the ACT table tanh (invert with arctanh), columns [A:SC] through
    the degree-7 DVE poly (invert with Newton; arctanh is a ~2e-3 initial
    guess so 3 iterations reach fp32 roundoff)."""
    if A is None:
        A = ACT_COLS
    t = np.asarray(t16, np.float32)
    r = np.empty_like(t)
    lim = np.float32(0.9999997)
    r[:, :A] = np.arctanh(np.clip(t[:, :A], -lim, lim))
    td = t[:, A:]
    y = np.arctanh(np.clip(td, -lim, lim))
    c0, c1, c2, c3 = [np.float32(c) for c in CH]
    three, five, seven = np.float32(3), np.float32(5), np.float32(7)
    for _ in range(3):
        s = y * y
        f = y * (((c3 * s + c2) * s + c1) * s + c0) - td
        df = ((seven * c3 * s + five * c2) * s + three * c1) * s + c0
        y = y - f / df
    r[:, A:] = y
    return r


def _make_in_map(inputs, xc):
    c16, c32 = _build_weight_blocks(
        inputs["Wf"], inputs["bf"], inputs["Wm"], inputs["bm"],
        inputs["Wu"], inputs["bu"], inputs["Wr"], inputs["br"],
    )
    return {"x_prep": _prep_core_x(xc), "consts16": c16, "consts32": c32}


def _run(inputs, trace=False):
    from concourse.bass_utils import run_bass_kernel_spmd

    x = np.asarray(inputs["x"], np.float32).reshape(B, NODES * F_IN)
    c16, c32 = _build_weight_blocks(
        inputs["Wf"], inputs["bf"], inputs["Wm"], inputs["bm"],
        inputs["Wu"], inputs["bu"], inputs["Wr"], inputs["br"],
    )

    nc = _build_program()

    in_maps = []
    for c in range(N_CORES):
        xc = x[c * B_CORE:(c + 1) * B_CORE]
        in_maps.append({
            "x_prep": _prep_core_x(xc),
            "consts16": c16,
            "consts32": c32,
        })

    res = run_bass_kernel_spmd(nc, in_maps, list(range(N_CORES)), trace=trace)
    outs = [_invert_r(res.results[c]["out"]).reshape(B_CORE)
            for c in range(N_CORES)]
    full = np.concatenate(outs).reshape(B, 1).astype(np.float32)
    return full, res


def kernel(**inputs):
    full, _ = _run(inputs, trace=False)
    return full
